# revision 2
# baseline (speedup 1.0000x reference)
"""Single-head causal attention on 8 Trainium2 NeuronCores.

Problem: x[8, 4096, 384], Wq/Wk/Wv[384, 64] ->
    out[b] = softmax(causal((x[b]Wq)(x[b]Wk)^T / sqrt(384))) @ (x[b]Wv)

Sharding: data-parallel over batch — core i computes batch element i.
Weights are replicated to every core.

Per-core kernel layout (all matmuls contract over the partition axis):
  - X^T tiles [c=128, t] are built from natural x tiles via PE transposes.
  - Q^T, K^T [64, T] = W^T X^T  (lhsT = W chunk [128c, 64], rhs = X^T).
    Both are stored twice (partitions 0:64 and 64:128) so score matmuls
    can be row-packed two-at-a-time into the 128x128 PE array.
  - V_ext [t=128, 65] = [X Wv | 1]  (ones column -> softmax denominator).
  - Scores are computed TRANSPOSED: S^T[s, q] = K Q^T so that the
    softmax sum over s becomes a matmul-friendly partition axis and
    P^T tiles feed the PV matmul with no per-tile transposes:
        O^T[h+1, q] += V_ext[s,:]^T @ P^T[s, q]   (row 64 = sum_s P)
  - exp via ScalarE activation (no max subtraction: |scores/sqrt(C)| is
    small for this distribution, exp cannot overflow in fp32).
  - Causal masking: multiply diagonal-block P^T tiles by one of four
    precomputed 0/1 masks (built once with gpsimd.affine_select).
  - O^T is PE-transposed back to [q=128, 65]; column 64 holds the row
    sums; divide and DMA out.
"""

import sys

if "/opt/trn_rl_repo" not in sys.path:
    sys.path.insert(0, "/opt/trn_rl_repo")

import numpy as np

import concourse.bass as bass  # noqa: F401  (AP types used implicitly)
import concourse.tile as tile
from concourse import bacc, mybir
from concourse.bass import ds
from concourse.bass_utils import run_bass_kernel_spmd
from concourse.masks import make_identity

B = 8
T_FULL = 4096
C = 384
H = 64
P = 128
TQ = 512  # q-block width
SCALE = 1.0 / float(np.sqrt(C))
F32 = mybir.dt.float32
F32R = mybir.dt.float32r

MM_DTYPE = F32R  # fast single-pass fp32 matmul mode
ROW_PACK = True  # run score matmuls two-at-a-time in PE row halves


def build_nc(T=T_FULL, mm_dtype=MM_DTYPE, row_pack=ROW_PACK):
    """Build the per-core Bass program (same program on all 8 cores)."""
    NT = T // P  # number of 128-row s-chunks
    NQ = T // TQ  # number of 512-row q-blocks
    CC = C // P  # 3 embed chunks
    SUB = TQ // P  # 4 sub-tiles per block

    def mm_cast(ap):
        return ap.bitcast(mm_dtype) if mm_dtype != F32 else ap

    nc = bacc.Bacc(
        "TRN2",
        target_bir_lowering=False,
        debug=False,
        enable_asserts=True,
        num_devices=B,
    )
    x_ap = nc.dram_tensor("x", [T, C], F32, kind="ExternalInput").ap()
    wq_ap = nc.dram_tensor("Wq", [C, H], F32, kind="ExternalInput").ap()
    wk_ap = nc.dram_tensor("Wk", [C, H], F32, kind="ExternalInput").ap()
    wv_ap = nc.dram_tensor("Wv", [C, H], F32, kind="ExternalInput").ap()
    out_ap = nc.dram_tensor("out", [T, H], F32, kind="ExternalOutput").ap()

    x_re = x_ap.rearrange("(n p) c -> p n c", p=P)  # [128, NT, 384]
    out_re = out_ap.rearrange("(n p) h -> p n h", p=P)  # [128, NT, 64]

    with tile.TileContext(nc) as tc:
        with (
            tc.tile_pool(name="consts", bufs=1) as consts,
            tc.tile_pool(name="xnat", bufs=3) as xnat,
            tc.tile_pool(name="xtp", bufs=3) as xtp,
            tc.tile_pool(name="qkt", bufs=1) as qktp,
            tc.tile_pool(name="vextp", bufs=1) as vextp,
            tc.tile_pool(name="ptp", bufs=3) as ptp,
            tc.tile_pool(name="otp", bufs=2) as otp,
            tc.tile_pool(name="op", bufs=2) as op_,
            tc.tile_pool(name="rvp", bufs=2) as rvp,
            tc.tile_pool(name="psum", bufs=2, space="PSUM") as psum,
        ):
            ident = consts.tile([P, P], F32)
            make_identity(nc, ident)
            wq_sb = consts.tile([P, CC, H], F32)
            nc.sync.dma_start(out=wq_sb, in_=wq_ap.rearrange("(c p) h -> p c h", p=P))
            wk_sb = consts.tile([P, CC, H], F32)
            nc.sync.dma_start(out=wk_sb, in_=wk_ap.rearrange("(c p) h -> p c h", p=P))
            wv_sb = consts.tile([P, CC, H], F32)
            nc.sync.dma_start(out=wv_sb, in_=wv_ap.rearrange("(c p) h -> p c h", p=P))

            # masks[d][s_local, q_local] = 1.0 where q_local - s_local - 128*d >= 0
            masks = consts.tile([P, SUB, TQ], F32)
            nc.vector.memset(masks, 1.0)
            for d in range(SUB):
                nc.gpsimd.affine_select(
                    out=masks[:, d, :],
                    in_=masks[:, d, :],
                    compare_op=mybir.AluOpType.is_ge,
                    fill=0.0,
                    base=-P * d,
                    pattern=[[1, TQ]],
                    channel_multiplier=-1,
                )

            if row_pack:
                # qt2: Q^T duplicated in both partition halves.
                # kt2: K^T chunk c lives at partitions 64*(c%2), col (c//2)*128.
                qt2 = qktp.tile([P, T], F32, tag="qt")
                kt2 = qktp.tile([P, (NT // 2) * P], F32, tag="kt")
            else:
                qt2 = qktp.tile([H, T], F32, tag="qt")
                kt2 = qktp.tile([H, T], F32, tag="kt")
            vext = vextp.tile([P, NT, H + 1], F32)
            nc.vector.memset(vext[:, :, H : H + 1], 1.0)

            def phase1(j):
                """Load x rows [512j, 512j+512), produce X^T, Q^T, K^T, V."""
                xn = xnat.tile([P, SUB, C], F32, name=f"xn{j}")
                nc.sync.dma_start(out=xn, in_=x_re[:, SUB * j : SUB * (j + 1), :])
                xt = xtp.tile([P, CC, TQ], F32, name=f"xt{j}")
                for st in range(SUB):
                    pst = psum.tile([P, CC, P], F32, tag="small", name=f"pst{j}_{st}")
                    for c in range(CC):
                        nc.tensor.transpose(
                            pst[:, c, :], xn[:, st, c * P : (c + 1) * P], ident
                        )
                    nc.vector.tensor_copy(
                        out=xt[:, :, st * P : (st + 1) * P], in_=pst
                    )
                psq = psum.tile([H, TQ], F32, tag="wide", name=f"psq{j}")
                psk = psum.tile([H, TQ], F32, tag="wide", name=f"psk{j}")
                for c in range(CC):
                    nc.tensor.matmul(
                        psq,
                        lhsT=mm_cast(wq_sb[:, c, :]),
                        rhs=mm_cast(xt[:, c, :]),
                        start=(c == 0),
                        stop=(c == CC - 1),
                    )
                for c in range(CC):
                    nc.tensor.matmul(
                        psk,
                        lhsT=mm_cast(wk_sb[:, c, :]),
                        rhs=mm_cast(xt[:, c, :]),
                        start=(c == 0),
                        stop=(c == CC - 1),
                    )
                blk = ds(j * TQ, TQ)
                if row_pack:
                    nc.vector.tensor_copy(out=qt2[0:H, blk], in_=psq)
                    nc.vector.tensor_copy(out=qt2[H:P, blk], in_=psq)
                    for st in range(SUB):
                        c = SUB * j + st
                        half = H * (c % 2)
                        nc.vector.tensor_copy(
                            out=kt2[half : half + H, (c // 2) * P : (c // 2 + 1) * P],
                            in_=psk[:, st * P : (st + 1) * P],
                        )
                else:
                    nc.vector.tensor_copy(out=qt2[:, blk], in_=psq)
                    nc.vector.tensor_copy(out=kt2[:, blk], in_=psk)
                psv = psum.tile([P, SUB, H], F32, tag="acc", name=f"psv{j}")
                for st in range(SUB):
                    for c in range(CC):
                        nc.tensor.matmul(
                            psv[:, st, :],
                            lhsT=mm_cast(xt[:, c, st * P : (st + 1) * P]),
                            rhs=mm_cast(wv_sb[:, c, :]),
                            start=(c == 0),
                            stop=(c == CC - 1),
                        )
                nc.vector.tensor_copy(
                    out=vext[:, SUB * j : SUB * (j + 1), 0:H], in_=psv
                )

            def phase2(j):
                """Attention for q rows [512j, 512j+512)."""
                nchunks = (j + 1) * SUB
                q_sl = ds(j * TQ, TQ)
                pso = psum.tile([H + 1, TQ], F32, tag="acc", name=f"pso{j}")
                for pr in range(nchunks // 2):
                    pss = psum.tile([P, 2 * TQ], F32, tag="wide", name=f"pss{j}_{pr}")
                    for h2 in range(2):
                        c = 2 * pr + h2
                        if row_pack:
                            half = H * (c % 2)
                            nc.tensor.matmul(
                                pss[:, h2 * TQ : (h2 + 1) * TQ],
                                lhsT=mm_cast(
                                    kt2[half : half + H, (c // 2) * P : (c // 2 + 1) * P]
                                ),
                                rhs=mm_cast(qt2[half : half + H, q_sl]),
                                start=True,
                                stop=True,
                                tile_position=(half, 0),
                            )
                        else:
                            nc.tensor.matmul(
                                pss[:, h2 * TQ : (h2 + 1) * TQ],
                                lhsT=mm_cast(kt2[:, c * P : (c + 1) * P]),
                                rhs=mm_cast(qt2[:, q_sl]),
                                start=True,
                                stop=True,
                            )
                    pt = ptp.tile([P, 2 * TQ], F32, name=f"pt{j}_{pr}")
                    nc.scalar.activation(
                        out=pt,
                        in_=pss,
                        func=mybir.ActivationFunctionType.Exp,
                        scale=SCALE,
                    )
                    for h2 in range(2):
                        c = 2 * pr + h2
                        d = c - SUB * j
                        if d >= 0:
                            nc.vector.tensor_mul(
                                out=pt[:, h2 * TQ : (h2 + 1) * TQ],
                                in0=pt[:, h2 * TQ : (h2 + 1) * TQ],
                                in1=masks[:, d, :],
                            )
                    for h2 in range(2):
                        c = 2 * pr + h2
                        nc.tensor.matmul(
                            pso,
                            lhsT=mm_cast(vext[:, c, :]),
                            rhs=mm_cast(pt[:, h2 * TQ : (h2 + 1) * TQ]),
                            start=(c == 0),
                            stop=(c == nchunks - 1),
                        )
                ot = otp.tile([H + 1, TQ], F32, name=f"ot{j}")
                nc.vector.tensor_copy(out=ot, in_=pso)
                pstr = psum.tile([P, SUB, H + 1], F32, tag="small", name=f"pstr{j}")
                for i in range(SUB):
                    nc.tensor.transpose(
                        pstr[:, i, :],
                        ot[:, i * P : (i + 1) * P],
                        ident[0 : H + 1, 0 : H + 1],
                    )
                o = op_.tile([P, SUB, H + 1], F32, name=f"o{j}")
                nc.vector.tensor_copy(out=o, in_=pstr)
                rv = rvp.tile([P, SUB], F32, name=f"rv{j}")
                nc.vector.reciprocal(out=rv, in_=o[:, :, H : H + 1])
                for i in range(SUB):
                    nc.vector.tensor_scalar_mul(
                        out=o[:, i, 0:H],
                        in0=o[:, i, 0:H],
                        scalar1=rv[:, i : i + 1],
                    )
                nc.sync.dma_start(
                    out=out_re[:, SUB * j : SUB * (j + 1), :], in_=o[:, :, 0:H]
                )

            # Interleave: phase2(j) only needs phase1(0..j), so emitting
            # phase1(j+1) between attention blocks lets the PE fill the
            # gaps left while ScalarE (exp) is the bottleneck.
            phase1(0)
            phase1(1)
            for j in range(NQ):
                if j + 2 < NQ:
                    phase1(j + 2)
                phase2(j)

    nc.compile()
    return nc


_NC_CACHE = {}


def _get_nc():
    if "nc" not in _NC_CACHE:
        _NC_CACHE["nc"] = build_nc()
    return _NC_CACHE["nc"]


def kernel(x, Wk, Wq, Wv, _trace=False, _trace_kwargs=None):
    x = np.ascontiguousarray(x, dtype=np.float32)
    Wk = np.ascontiguousarray(Wk, dtype=np.float32)
    Wq = np.ascontiguousarray(Wq, dtype=np.float32)
    Wv = np.ascontiguousarray(Wv, dtype=np.float32)
    nc = _get_nc()
    in_maps = [
        {"x": x[b], "Wq": Wq, "Wk": Wk, "Wv": Wv} for b in range(B)
    ]
    res = run_bass_kernel_spmd(
        nc, in_maps, list(range(B)), trace=_trace, **(_trace_kwargs or {})
    )
    out = np.stack([res.results[b]["out"] for b in range(B)], axis=0)
    if _trace:
        return out, res
    return out


# revision 7
# speedup vs baseline: 1.7310x; 1.7310x over previous
"""Single-head causal attention on 8 Trainium2 NeuronCores.

Problem: x[8, 4096, 384], Wq/Wk/Wv[384, 64] ->
    out[b] = softmax(causal((x[b]Wq)(x[b]Wk)^T / sqrt(384))) @ (x[b]Wv)

Sharding: data-parallel over batch — core i computes batch element i.
Weights are replicated to every core.

Per-core kernel layout (all matmuls contract over the partition axis):
  - X^T tiles [c=128, t] are built from natural x tiles via PE transposes.
  - Q^T, K^T [64, T] = W^T X^T  (lhsT = W chunk [128c, 64], rhs = X^T).
    Both are stored twice (partitions 0:64 and 64:128) so score matmuls
    can be row-packed two-at-a-time into the 128x128 PE array.
  - V_ext [t=128, 65] = [X Wv | 1]  (ones column -> softmax denominator).
  - Scores are computed TRANSPOSED: S^T[s, q] = K Q^T so that the
    softmax sum over s becomes a matmul-friendly partition axis and
    P^T tiles feed the PV matmul with no per-tile transposes:
        O^T[h+1, q] += V_ext[s,:]^T @ P^T[s, q]   (row 64 = sum_s P)
  - exp via ScalarE activation (no max subtraction: |scores/sqrt(C)| is
    small for this distribution, exp cannot overflow in fp32).
  - Causal masking: multiply diagonal-block P^T tiles by one of four
    precomputed 0/1 masks (built once with gpsimd.affine_select).
  - O^T is PE-transposed back to [q=128, 65]; column 64 holds the row
    sums; divide and DMA out.
"""

import sys

if "/opt/trn_rl_repo" not in sys.path:
    sys.path.insert(0, "/opt/trn_rl_repo")

import numpy as np

import concourse.bass as bass  # noqa: F401  (AP types used implicitly)
import concourse.tile as tile
from concourse import bacc, mybir
from concourse.bass import ds
from concourse.bass_utils import run_bass_kernel_spmd
from concourse.masks import make_identity

B = 8
T_FULL = 4096
C = 384
H = 64
P = 128
TQ = 512  # q-block width
SCALE = 1.0 / float(np.sqrt(C))
F32 = mybir.dt.float32
F32R = mybir.dt.float32r

MM_DTYPE = F32R  # fast single-pass fp32 matmul mode
ROW_PACK = True  # run score matmuls two-at-a-time in PE row halves


def build_nc(T=T_FULL, mm_dtype=MM_DTYPE, row_pack=ROW_PACK):
    """Build the per-core Bass program (same program on all 8 cores)."""
    NT = T // P  # number of 128-row s-chunks
    NQ = T // TQ  # number of 512-row q-blocks
    CC = C // P  # 3 embed chunks
    SUB = TQ // P  # 4 sub-tiles per block

    MMD = mm_dtype  # tiles feeding matmuls are allocated in this dtype

    def mm_cast(ap):
        return ap

    nc = bacc.Bacc(
        "TRN2",
        target_bir_lowering=False,
        debug=False,
        enable_asserts=True,
        num_devices=B,
    )
    x_ap = nc.dram_tensor("x", [T, C], F32, kind="ExternalInput").ap()
    wq_ap = nc.dram_tensor("Wq", [C, H], F32, kind="ExternalInput").ap()
    wk_ap = nc.dram_tensor("Wk", [C, H], F32, kind="ExternalInput").ap()
    wv_ap = nc.dram_tensor("Wv", [C, H], F32, kind="ExternalInput").ap()
    out_ap = nc.dram_tensor("out", [T, H], F32, kind="ExternalOutput").ap()

    x_re = x_ap.rearrange("(n p) c -> p n c", p=P)  # [128, NT, 384]
    out_re = out_ap.rearrange("(n p) h -> p n h", p=P)  # [128, NT, 64]

    with tile.TileContext(nc) as tc:
        with (
            tc.tile_pool(name="consts", bufs=1) as consts,
            tc.tile_pool(name="xnat", bufs=3) as xnat,
            tc.tile_pool(name="xtp", bufs=3) as xtp,
            tc.tile_pool(name="qkt", bufs=1) as qktp,
            tc.tile_pool(name="vextp", bufs=1) as vextp,
            tc.tile_pool(name="ptp", bufs=3) as ptp,
            tc.tile_pool(name="otp", bufs=2) as otp,
            tc.tile_pool(name="op", bufs=2) as op_,
            tc.tile_pool(name="rvp", bufs=2) as rvp,
            tc.tile_pool(name="psum", bufs=2, space="PSUM") as psum,
        ):
            ident = consts.tile([P, P], F32)
            make_identity(nc, ident)
            wq_sb = consts.tile([P, CC, H], MMD)
            nc.gpsimd.dma_start(out=wq_sb, in_=wq_ap.rearrange("(c p) h -> p c h", p=P))
            wk_sb = consts.tile([P, CC, H], MMD)
            nc.gpsimd.dma_start(out=wk_sb, in_=wk_ap.rearrange("(c p) h -> p c h", p=P))
            wv_sb = consts.tile([P, CC, H], MMD)
            nc.gpsimd.dma_start(out=wv_sb, in_=wv_ap.rearrange("(c p) h -> p c h", p=P))

            # masks[d][s_local, q_local] = 1.0 where q_local - s_local - 128*d >= 0
            masks = consts.tile([P, SUB, TQ], F32)
            nc.vector.memset(masks, 1.0)
            for d in range(SUB):
                nc.gpsimd.affine_select(
                    out=masks[:, d, :],
                    in_=masks[:, d, :],
                    compare_op=mybir.AluOpType.is_ge,
                    fill=0.0,
                    base=-P * d,
                    pattern=[[1, TQ]],
                    channel_multiplier=-1,
                )

            if row_pack:
                # qt2: Q^T duplicated in both partition halves.
                # kt2: K^T chunk c lives at partitions 64*(c%2), col (c//2)*128.
                qt2 = qktp.tile([P, T], MMD, tag="qt")
                kt2 = qktp.tile([P, (NT // 2) * P], MMD, tag="kt")
            else:
                qt2 = qktp.tile([H, T], MMD, tag="qt")
                kt2 = qktp.tile([H, T], MMD, tag="kt")
            vext = vextp.tile([P, NT, H + 1], MMD)
            ones_col = consts.tile([P, NT, 1], F32)
            nc.vector.memset(ones_col, 1.0)
            nc.vector.tensor_copy(out=vext[:, :, H : H + 1], in_=ones_col)

            def phase1(j):
                """Load x rows [512j, 512j+512), produce X^T, Q^T, K^T, V."""
                xn = xnat.tile([P, SUB, C], F32, tag="xn", name=f"xn{j}")
                nc.sync.dma_start(out=xn, in_=x_re[:, SUB * j : SUB * (j + 1), :])
                xt = xtp.tile([P, CC, TQ], MMD, tag="xt", name=f"xt{j}")
                for st in range(SUB):
                    pst = psum.tile([P, CC, P], F32, tag="small", name=f"pst{j}_{st}")
                    for c in range(CC):
                        nc.tensor.transpose(
                            pst[:, c, :], xn[:, st, c * P : (c + 1) * P], ident
                        )
                    nc.vector.tensor_copy(
                        out=xt[:, :, st * P : (st + 1) * P], in_=pst
                    )
                psq = psum.tile([H, TQ], F32, tag="wide", name=f"psq{j}")
                psk = psum.tile([H, TQ], F32, tag="wide", name=f"psk{j}")
                for c in range(CC):
                    nc.tensor.matmul(
                        psq,
                        lhsT=mm_cast(wq_sb[:, c, :]),
                        rhs=mm_cast(xt[:, c, :]),
                        start=(c == 0),
                        stop=(c == CC - 1),
                    )
                for c in range(CC):
                    nc.tensor.matmul(
                        psk,
                        lhsT=mm_cast(wk_sb[:, c, :]),
                        rhs=mm_cast(xt[:, c, :]),
                        start=(c == 0),
                        stop=(c == CC - 1),
                    )
                blk = ds(j * TQ, TQ)
                if row_pack:
                    nc.vector.tensor_copy(out=qt2[0:H, blk], in_=psq)
                    nc.vector.tensor_copy(out=qt2[H:P, blk], in_=psq)
                    for st in range(SUB):
                        c = SUB * j + st
                        half = H * (c % 2)
                        nc.vector.tensor_copy(
                            out=kt2[half : half + H, (c // 2) * P : (c // 2 + 1) * P],
                            in_=psk[:, st * P : (st + 1) * P],
                        )
                else:
                    nc.vector.tensor_copy(out=qt2[:, blk], in_=psq)
                    nc.vector.tensor_copy(out=kt2[:, blk], in_=psk)
                psv = psum.tile([P, SUB, H], F32, tag="acc", name=f"psv{j}")
                for st in range(SUB):
                    for c in range(CC):
                        nc.tensor.matmul(
                            psv[:, st, :],
                            lhsT=mm_cast(xt[:, c, st * P : (st + 1) * P]),
                            rhs=mm_cast(wv_sb[:, c, :]),
                            start=(c == 0),
                            stop=(c == CC - 1),
                        )
                nc.vector.tensor_copy(
                    out=vext[:, SUB * j : SUB * (j + 1), 0:H], in_=psv
                )

            def phase2(j):
                """Attention for q rows [512j, 512j+512)."""
                nchunks = (j + 1) * SUB
                q_sl = ds(j * TQ, TQ)
                pso = psum.tile([H + 1, TQ], F32, tag="acc", name=f"pso{j}")
                for pr in range(nchunks // 2):
                    pss = psum.tile([P, 2 * TQ], F32, tag="wide", name=f"pss{j}_{pr}")
                    for h2 in range(2):
                        c = 2 * pr + h2
                        if row_pack:
                            half = H * (c % 2)
                            nc.tensor.matmul(
                                pss[:, h2 * TQ : (h2 + 1) * TQ],
                                lhsT=mm_cast(
                                    kt2[half : half + H, (c // 2) * P : (c // 2 + 1) * P]
                                ),
                                rhs=mm_cast(qt2[half : half + H, q_sl]),
                                start=True,
                                stop=True,
                                tile_position=(half, 0),
                            )
                        else:
                            nc.tensor.matmul(
                                pss[:, h2 * TQ : (h2 + 1) * TQ],
                                lhsT=mm_cast(kt2[:, c * P : (c + 1) * P]),
                                rhs=mm_cast(qt2[:, q_sl]),
                                start=True,
                                stop=True,
                            )
                    pt = ptp.tile([P, 2 * TQ], MMD, tag="pt", name=f"pt{j}_{pr}")
                    nc.scalar.activation(
                        out=pt,
                        in_=pss,
                        func=mybir.ActivationFunctionType.Exp,
                        scale=SCALE,
                    )
                    for h2 in range(2):
                        c = 2 * pr + h2
                        d = c - SUB * j
                        if d >= 0:
                            nc.vector.tensor_mul(
                                out=pt[:, h2 * TQ : (h2 + 1) * TQ],
                                in0=pt[:, h2 * TQ : (h2 + 1) * TQ],
                                in1=masks[:, d, :],
                            )
                    for h2 in range(2):
                        c = 2 * pr + h2
                        nc.tensor.matmul(
                            pso,
                            lhsT=mm_cast(vext[:, c, :]),
                            rhs=mm_cast(pt[:, h2 * TQ : (h2 + 1) * TQ]),
                            start=(c == 0),
                            stop=(c == nchunks - 1),
                        )
                ot = otp.tile([H + 1, TQ], F32, tag="ot", name=f"ot{j}")
                nc.vector.tensor_copy(out=ot, in_=pso)
                pstr = psum.tile([P, SUB, H + 1], F32, tag="small", name=f"pstr{j}")
                for i in range(SUB):
                    nc.tensor.transpose(
                        pstr[:, i, :],
                        ot[:, i * P : (i + 1) * P],
                        ident[0 : H + 1, 0 : H + 1],
                    )
                o = op_.tile([P, SUB, H + 1], F32, tag="o", name=f"o{j}")
                nc.vector.tensor_copy(out=o, in_=pstr)
                rv = rvp.tile([P, SUB], F32, tag="rv", name=f"rv{j}")
                nc.vector.reciprocal(out=rv, in_=o[:, :, H : H + 1])
                for i in range(SUB):
                    nc.vector.tensor_scalar_mul(
                        out=o[:, i, 0:H],
                        in0=o[:, i, 0:H],
                        scalar1=rv[:, i : i + 1],
                    )
                nc.sync.dma_start(
                    out=out_re[:, SUB * j : SUB * (j + 1), :], in_=o[:, :, 0:H]
                )

            # Interleave: phase2(j) only needs phase1(0..j), so emitting
            # phase1(j+1) between attention blocks lets the PE fill the
            # gaps left while ScalarE (exp) is the bottleneck.
            for j in range(min(2, NQ)):
                phase1(j)
            for j in range(NQ):
                if j + 2 < NQ:
                    phase1(j + 2)
                phase2(j)

    nc.compile()
    return nc


_NC_CACHE = {}


def _get_nc():
    if "nc" not in _NC_CACHE:
        _NC_CACHE["nc"] = build_nc()
    return _NC_CACHE["nc"]


def kernel(x, Wk, Wq, Wv, _trace=False, _trace_kwargs=None):
    x = np.ascontiguousarray(x, dtype=np.float32)
    Wk = np.ascontiguousarray(Wk, dtype=np.float32)
    Wq = np.ascontiguousarray(Wq, dtype=np.float32)
    Wv = np.ascontiguousarray(Wv, dtype=np.float32)
    nc = _get_nc()
    in_maps = [
        {"x": x[b], "Wq": Wq, "Wk": Wk, "Wv": Wv} for b in range(B)
    ]
    res = run_bass_kernel_spmd(
        nc, in_maps, list(range(B)), trace=_trace, **(_trace_kwargs or {})
    )
    out = np.stack([res.results[b]["out"] for b in range(B)], axis=0)
    if _trace:
        return out, res
    return out


# revision 9
# speedup vs baseline: 1.9844x; 1.1464x over previous
"""Single-head causal attention on 8 Trainium2 NeuronCores.

Problem: x[8, 4096, 384], Wq/Wk/Wv[384, 64] ->
    out[b] = softmax(causal((x[b]Wq)(x[b]Wk)^T / sqrt(384))) @ (x[b]Wv)

Sharding: data-parallel over batch — core i computes batch element i.
Weights are replicated to every core.

Per-core kernel layout (all matmuls contract over the partition axis):
  - X^T tiles [c=128, t] are built from natural x tiles via PE transposes.
  - Q^T, K^T [64, T] = W^T X^T  (lhsT = W chunk [128c, 64], rhs = X^T).
    Both are stored twice (partitions 0:64 and 64:128) so score matmuls
    can be row-packed two-at-a-time into the 128x128 PE array.
  - V_ext [t=128, 65] = [X Wv | 1]  (ones column -> softmax denominator).
  - Scores are computed TRANSPOSED: S^T[s, q] = K Q^T so that the
    softmax sum over s becomes a matmul-friendly partition axis and
    P^T tiles feed the PV matmul with no per-tile transposes:
        O^T[h+1, q] += V_ext[s,:]^T @ P^T[s, q]   (row 64 = sum_s P)
  - exp via ScalarE activation (no max subtraction: |scores/sqrt(C)| is
    small for this distribution, exp cannot overflow in fp32).
  - Causal masking: multiply diagonal-block P^T tiles by one of four
    precomputed 0/1 masks (built once with gpsimd.affine_select).
  - O^T is PE-transposed back to [q=128, 65]; column 64 holds the row
    sums; divide and DMA out.
"""

import sys

if "/opt/trn_rl_repo" not in sys.path:
    sys.path.insert(0, "/opt/trn_rl_repo")

import numpy as np

import concourse.bass as bass  # noqa: F401  (AP types used implicitly)
import concourse.tile as tile
from concourse import bacc, mybir
from concourse.bass import ds
from concourse.bass_utils import run_bass_kernel_spmd
from concourse.masks import make_identity

B = 8
T_FULL = 4096
C = 384
H = 64
P = 128
TQ = 512  # q-block width
SCALE = 1.0 / float(np.sqrt(C))
F32 = mybir.dt.float32
F32R = mybir.dt.float32r

F16 = mybir.dt.float16
MM_DTYPE = F16  # matmul pipeline dtype (fp16: 1 cyc/row + fast weight load)
ROW_PACK = True  # run score matmuls two-at-a-time in PE row halves


def build_nc(T=T_FULL, mm_dtype=MM_DTYPE, row_pack=ROW_PACK):
    """Build the per-core Bass program (same program on all 8 cores)."""
    NT = T // P  # number of 128-row s-chunks
    NQ = T // TQ  # number of 512-row q-blocks
    CC = C // P  # 3 embed chunks
    SUB = TQ // P  # 4 sub-tiles per block

    MMD = mm_dtype  # tiles feeding matmuls are allocated in this dtype

    def mm_cast(ap):
        return ap

    nc = bacc.Bacc(
        "TRN2",
        target_bir_lowering=False,
        debug=False,
        enable_asserts=True,
        num_devices=B,
    )
    x_ap = nc.dram_tensor("x", [T, C], F32, kind="ExternalInput").ap()
    wq_ap = nc.dram_tensor("Wq", [C, H], F32, kind="ExternalInput").ap()
    wk_ap = nc.dram_tensor("Wk", [C, H], F32, kind="ExternalInput").ap()
    wv_ap = nc.dram_tensor("Wv", [C, H], F32, kind="ExternalInput").ap()
    out_ap = nc.dram_tensor("out", [T, H], F32, kind="ExternalOutput").ap()

    x_re = x_ap.rearrange("(n p) c -> p n c", p=P)  # [128, NT, 384]
    out_re = out_ap.rearrange("(n p) h -> p n h", p=P)  # [128, NT, 64]

    with tile.TileContext(nc) as tc:
        with (
            tc.tile_pool(name="consts", bufs=1) as consts,
            tc.tile_pool(name="xnat", bufs=3) as xnat,
            tc.tile_pool(name="xtp", bufs=3) as xtp,
            tc.tile_pool(name="qkt", bufs=1) as qktp,
            tc.tile_pool(name="vextp", bufs=1) as vextp,
            tc.tile_pool(name="ptp", bufs=3) as ptp,
            tc.tile_pool(name="otp", bufs=2) as otp,
            tc.tile_pool(name="vtp", bufs=2) as vtp,
            tc.tile_pool(name="op", bufs=2) as op_,
            tc.tile_pool(name="rvp", bufs=2) as rvp,
            tc.tile_pool(name="psum", bufs=2, space="PSUM") as psum,
        ):
            ident = consts.tile([P, P], F32)
            make_identity(nc, ident)
            ident_h = consts.tile([P, P], MMD)
            make_identity(nc, ident_h)
            wq_sb = consts.tile([P, CC, H], MMD)
            nc.gpsimd.dma_start(out=wq_sb, in_=wq_ap.rearrange("(c p) h -> p c h", p=P))
            wk_sb = consts.tile([P, CC, H], MMD)
            nc.gpsimd.dma_start(out=wk_sb, in_=wk_ap.rearrange("(c p) h -> p c h", p=P))
            wv_sb = consts.tile([P, CC, H], MMD)
            nc.gpsimd.dma_start(out=wv_sb, in_=wv_ap.rearrange("(c p) h -> p c h", p=P))

            # masks[d][s_local, q_local] = 1.0 where q_local - s_local - 128*d >= 0
            masks = consts.tile([P, SUB, TQ], MMD)
            nc.vector.memset(masks, 1.0)
            for d in range(SUB):
                nc.gpsimd.affine_select(
                    out=masks[:, d, :],
                    in_=masks[:, d, :],
                    compare_op=mybir.AluOpType.is_ge,
                    fill=0.0,
                    base=-P * d,
                    pattern=[[1, TQ]],
                    channel_multiplier=-1,
                )

            if row_pack:
                # qt2: Q^T duplicated in both partition halves.
                # kt2: K^T chunk c lives at partitions 64*(c%2), col (c//2)*128.
                qt2 = qktp.tile([P, T], MMD, tag="qt")
                kt2 = qktp.tile([P, (NT // 2) * P], MMD, tag="kt")
            else:
                qt2 = qktp.tile([H, T], MMD, tag="qt")
                kt2 = qktp.tile([H, T], MMD, tag="kt")
            vext = vextp.tile([P, NT, H + 1], MMD)
            ones_col = consts.tile([P, NT, 1], F32)
            nc.vector.memset(ones_col, 1.0)
            nc.vector.tensor_copy(out=vext[:, :, H : H + 1], in_=ones_col)

            def phase1(j):
                """Load x rows [512j, 512j+512), produce X^T, Q^T, K^T, V."""
                xn = xnat.tile([P, SUB, C], MMD, tag="xn", name=f"xn{j}")
                nc.gpsimd.dma_start(out=xn, in_=x_re[:, SUB * j : SUB * (j + 1), :])
                xt = xtp.tile([P, CC, TQ], MMD, tag="xt", name=f"xt{j}")
                for st in range(SUB):
                    pst = psum.tile([P, CC, P], MMD, tag="small", name=f"pst{j}_{st}")
                    for c in range(CC):
                        nc.tensor.transpose(
                            pst[:, c, :], xn[:, st, c * P : (c + 1) * P], ident_h
                        )
                    nc.vector.tensor_copy(
                        out=xt[:, :, st * P : (st + 1) * P], in_=pst
                    )
                psq = psum.tile([H, TQ], F32, tag="wide", name=f"psq{j}")
                psk = psum.tile([H, TQ], F32, tag="wide", name=f"psk{j}")
                for c in range(CC):
                    nc.tensor.matmul(
                        psq,
                        lhsT=mm_cast(wq_sb[:, c, :]),
                        rhs=mm_cast(xt[:, c, :]),
                        start=(c == 0),
                        stop=(c == CC - 1),
                    )
                for c in range(CC):
                    nc.tensor.matmul(
                        psk,
                        lhsT=mm_cast(wk_sb[:, c, :]),
                        rhs=mm_cast(xt[:, c, :]),
                        start=(c == 0),
                        stop=(c == CC - 1),
                    )
                blk = ds(j * TQ, TQ)
                if row_pack:
                    nc.vector.tensor_copy(out=qt2[0:H, blk], in_=psq)
                    nc.vector.tensor_copy(out=qt2[H:P, blk], in_=psq)
                    for st in range(SUB):
                        c = SUB * j + st
                        half = H * (c % 2)
                        nc.vector.tensor_copy(
                            out=kt2[half : half + H, (c // 2) * P : (c // 2 + 1) * P],
                            in_=psk[:, st * P : (st + 1) * P],
                        )
                else:
                    nc.vector.tensor_copy(out=qt2[:, blk], in_=psq)
                    nc.vector.tensor_copy(out=kt2[:, blk], in_=psk)
                psv = psum.tile([H, TQ], F32, tag="wide", name=f"psv{j}")
                for c in range(CC):
                    nc.tensor.matmul(
                        psv,
                        lhsT=mm_cast(wv_sb[:, c, :]),
                        rhs=mm_cast(xt[:, c, :]),
                        start=(c == 0),
                        stop=(c == CC - 1),
                    )
                vt = vtp.tile([H, TQ], MMD, tag="vt", name=f"vt{j}")
                nc.vector.tensor_copy(out=vt, in_=psv)
                for st in range(SUB):
                    pvt = psum.tile([P, H], MMD, tag="small", name=f"pvt{j}_{st}")
                    nc.tensor.transpose(
                        pvt, vt[:, st * P : (st + 1) * P], ident_h[0:H, 0:H]
                    )
                    nc.vector.tensor_copy(
                        out=vext[:, SUB * j + st, 0:H], in_=pvt
                    )

            def phase2(j):
                """Attention for q rows [512j, 512j+512)."""
                nchunks = (j + 1) * SUB
                q_sl = ds(j * TQ, TQ)
                pso = psum.tile([H + 1, TQ], F32, tag="acc", name=f"pso{j}")
                for pr in range(nchunks // 2):
                    pss = psum.tile([P, 2 * TQ], F32, tag="wide", name=f"pss{j}_{pr}")
                    for h2 in range(2):
                        c = 2 * pr + h2
                        if row_pack:
                            half = H * (c % 2)
                            nc.tensor.matmul(
                                pss[:, h2 * TQ : (h2 + 1) * TQ],
                                lhsT=mm_cast(
                                    kt2[half : half + H, (c // 2) * P : (c // 2 + 1) * P]
                                ),
                                rhs=mm_cast(qt2[half : half + H, q_sl]),
                                start=True,
                                stop=True,
                                tile_position=(half, 0),
                            )
                        else:
                            nc.tensor.matmul(
                                pss[:, h2 * TQ : (h2 + 1) * TQ],
                                lhsT=mm_cast(kt2[:, c * P : (c + 1) * P]),
                                rhs=mm_cast(qt2[:, q_sl]),
                                start=True,
                                stop=True,
                            )
                    pt = ptp.tile([P, 2 * TQ], MMD, tag="pt", name=f"pt{j}_{pr}")
                    nc.scalar.activation(
                        out=pt,
                        in_=pss,
                        func=mybir.ActivationFunctionType.Exp,
                        scale=SCALE,
                    )
                    for h2 in range(2):
                        c = 2 * pr + h2
                        d = c - SUB * j
                        if d >= 0:
                            nc.vector.tensor_mul(
                                out=pt[:, h2 * TQ : (h2 + 1) * TQ],
                                in0=pt[:, h2 * TQ : (h2 + 1) * TQ],
                                in1=masks[:, d, :],
                            )
                    for h2 in range(2):
                        c = 2 * pr + h2
                        nc.tensor.matmul(
                            pso,
                            lhsT=mm_cast(vext[:, c, :]),
                            rhs=mm_cast(pt[:, h2 * TQ : (h2 + 1) * TQ]),
                            start=(c == 0),
                            stop=(c == nchunks - 1),
                        )
                ot = otp.tile([H + 1, TQ], F32, tag="ot", name=f"ot{j}")
                nc.vector.tensor_copy(out=ot, in_=pso)
                pstr = psum.tile([P, SUB, H + 1], F32, tag="small", name=f"pstr{j}")
                for i in range(SUB):
                    nc.tensor.transpose(
                        pstr[:, i, :],
                        ot[:, i * P : (i + 1) * P],
                        ident[0 : H + 1, 0 : H + 1],
                    )
                o = op_.tile([P, SUB, H + 1], F32, tag="o", name=f"o{j}")
                nc.vector.tensor_copy(out=o, in_=pstr)
                rv = rvp.tile([P, SUB], F32, tag="rv", name=f"rv{j}")
                nc.vector.reciprocal(out=rv, in_=o[:, :, H : H + 1])
                for i in range(SUB):
                    nc.vector.tensor_scalar_mul(
                        out=o[:, i, 0:H],
                        in0=o[:, i, 0:H],
                        scalar1=rv[:, i : i + 1],
                    )
                nc.sync.dma_start(
                    out=out_re[:, SUB * j : SUB * (j + 1), :], in_=o[:, :, 0:H]
                )

            # Interleave: phase2(j) only needs phase1(0..j), so emitting
            # phase1(j+1) between attention blocks lets the PE fill the
            # gaps left while ScalarE (exp) is the bottleneck.
            for j in range(min(2, NQ)):
                phase1(j)
            for j in range(NQ):
                if j + 2 < NQ:
                    phase1(j + 2)
                phase2(j)

    nc.compile()
    return nc


_NC_CACHE = {}


def _get_nc():
    if "nc" not in _NC_CACHE:
        _NC_CACHE["nc"] = build_nc()
    return _NC_CACHE["nc"]


def kernel(x, Wk, Wq, Wv, _trace=False, _trace_kwargs=None):
    x = np.ascontiguousarray(x, dtype=np.float32)
    Wk = np.ascontiguousarray(Wk, dtype=np.float32)
    Wq = np.ascontiguousarray(Wq, dtype=np.float32)
    Wv = np.ascontiguousarray(Wv, dtype=np.float32)
    nc = _get_nc()
    in_maps = [
        {"x": x[b], "Wq": Wq, "Wk": Wk, "Wv": Wv} for b in range(B)
    ]
    res = run_bass_kernel_spmd(
        nc, in_maps, list(range(B)), trace=_trace, **(_trace_kwargs or {})
    )
    out = np.stack([res.results[b]["out"] for b in range(B)], axis=0)
    if _trace:
        return out, res
    return out
